# revision 7
# baseline (speedup 1.0000x reference)
"""Trainium2 Bass kernel for a 2-layer dense-MoE PoC model.

Model (per reference):
  h = emb[x]                                         [B,T,D]
  for l in 0..L-1:
    xn = layernorm(h) * g + b
    rw = softmax(xn @ gate_w[l] + gate_b[l])         [B,T,E]
    e  = gelu_erf(einsum('btc,eci->btei', xn, w1[l]))
    eo = einsum('btei,eic->btec', e, w2[l])
    h  = h + einsum('btec,bte->btc', eo, rw)
  logits = h @ head_w + head_b
  return logits, stack(eo per layer)

Shapes: VOCAB=32000 D=1024 E=8 I=2048 L=2 B=4 T=2048.

Distribution: data-parallel over the 8192 tokens across 8 NeuronCores
(1024 tokens each); every core holds the full weights.  No collectives.

Numerics: all large matmuls run in float32r (fp32 rounded to an 11-bit
mantissa; full PE rate).  Weights are pre-rounded on host and staged as
f32r; activations are rounded on-device by the PSUM-evacuation copies
that produce the matmul operands.  Everything else is fp32.
"""
from contextlib import ExitStack

import numpy as np

import concourse.bass as bass
import concourse.mybir as mybir
import concourse.tile as tile
from concourse import bacc
from concourse.bass_utils import run_bass_kernel_spmd

# model dims (hardcoded per contract)
VOCAB, D, E, I, L, B, T = 32000, 1024, 8, 2048, 2, 4, 2048
LN_EPS = 1e-5
NCORES = 8
TOK = B * T              # 8192
TPC = TOK // NCORES      # 1024 tokens per core
G = TPC // 128           # 8 token tiles per core
DC = D // 128            # 8 feature chunks
IC = I // 128            # 16 expert-hidden chunks
NTC = TPC // 512         # 2 moving chunks of 512 tokens
DCB = D // 512           # 2 output D chunks of 512

F32 = mybir.dt.float32
F32R = mybir.dt.float32r
I32 = mybir.dt.int32
AF = mybir.ActivationFunctionType
ALU = mybir.AluOpType

# vocab chunks for the head: 62 x 512 + 1 x 256
VCHUNKS = [(i * 512, 512) for i in range(VOCAB // 512)]
if VOCAB % 512:
    VCHUNKS.append((512 * (VOCAB // 512), VOCAB % 512))


def round_f32r(x: np.ndarray) -> np.ndarray:
    """Round-to-nearest fp32 -> f32r (11 explicit mantissa bits)."""
    x = np.ascontiguousarray(x, dtype=np.float32)
    b = x.view(np.uint32)
    r = (b + np.uint32(0x800) + ((b >> np.uint32(12)) & np.uint32(1))) & np.uint32(0xFFFFF000)
    return r.view(np.float32)


def build():
    from concourse.masks import make_identity

    nc = bacc.Bacc("TRN2", target_bir_lowering=False, debug=False)

    ids_d = nc.declare_dram_parameter("ids", [TPC], I32, isOutput=False)
    emb_d = nc.declare_dram_parameter("emb", [VOCAB, D], F32, isOutput=False)
    lng_d = nc.declare_dram_parameter("ln_g", [L, D], F32, isOutput=False)
    lnb_d = nc.declare_dram_parameter("ln_b", [L, D], F32, isOutput=False)
    gw_d = nc.declare_dram_parameter("gate_w", [L, D, E], F32R, isOutput=False)
    gb_d = nc.declare_dram_parameter("gate_b", [L, E], F32, isOutput=False)
    w1_d = nc.declare_dram_parameter("w1", [L, E, D, I], F32R, isOutput=False)
    w2_d = nc.declare_dram_parameter("w2", [L, E, I, D], F32R, isOutput=False)
    hw_d = nc.declare_dram_parameter("head_w", [D, VOCAB], F32R, isOutput=False)
    hb_d = nc.declare_dram_parameter("head_b", [VOCAB], F32, isOutput=False)
    logits_d = nc.declare_dram_parameter("logits", [TPC, VOCAB], F32, isOutput=True)
    experts_d = nc.declare_dram_parameter("experts", [L, TPC, E, D], F32, isOutput=True)

    def bcast(dram_ap, parts, n):
        """DRAM AP replicating a length-n row across `parts` partitions."""
        return bass.AP(
            tensor=dram_ap.tensor,
            offset=dram_ap.offset,
            ap=[[0, parts]] + [list(p) for p in dram_ap.ap],
        )

    with ExitStack() as ctx:
        tc = ctx.enter_context(tile.TileContext(nc))

        persist = ctx.enter_context(tc.tile_pool(name="persist", bufs=1))
        small = ctx.enter_context(tc.tile_pool(name="small", bufs=4))
        ps_t = ctx.enter_context(tc.tile_pool(name="ps_t", bufs=2, space="PSUM"))

        ident = persist.tile([128, 128], F32)
        make_identity(nc, ident)
        eps_t = persist.tile([128, 1], F32)
        nc.vector.memset(eps_t, LN_EPS)

        h_t = persist.tile([128, G, D], F32)
        rw_t = persist.tile([128, G, E], F32)

        # ---- embedding gather ----
        with nc.named_scope("embed"):
            ids_t = persist.tile([128, G], I32)
            nc.sync.dma_start(out=ids_t, in_=ids_d.rearrange("(g p) -> p g", p=128))
            for g in range(G):
                nc.gpsimd.indirect_dma_start(
                    out=h_t[:, g, :],
                    out_offset=None,
                    in_=emb_d[:],
                    in_offset=bass.IndirectOffsetOnAxis(ap=ids_t[:, g : g + 1], axis=0),
                )

        with ExitStack() as lctx:
            lpool = lctx.enter_context(tc.tile_pool(name="lpool", bufs=1))
            xnp = lctx.enter_context(tc.tile_pool(name="xnp", bufs=2))
            w1p = lctx.enter_context(tc.tile_pool(name="w1p", bufs=2))
            w2p = lctx.enter_context(tc.tile_pool(name="w2p", bufs=2))
            evacp = lctx.enter_context(tc.tile_pool(name="evacp", bufs=3))
            ps_lg = lctx.enter_context(tc.tile_pool(name="ps_lg", bufs=1, space="PSUM"))
            ps_e1 = lctx.enter_context(tc.tile_pool(name="ps_e1", bufs=3, space="PSUM"))
            ps_eo = lctx.enter_context(tc.tile_pool(name="ps_eo", bufs=2, space="PSUM"))

            for l in range(L):
                with nc.named_scope(f"layer{l}"):
                    # per-layer params
                    g_sb = lpool.tile([128, DC, 1], F32, tag="g_sb")
                    b_sb = lpool.tile([128, DC, 1], F32, tag="b_sb")
                    nc.gpsimd.dma_start(out=g_sb, in_=lng_d[l].rearrange("(c p) -> p c ()", p=128))
                    nc.gpsimd.dma_start(out=b_sb, in_=lnb_d[l].rearrange("(c p) -> p c ()", p=128))
                    gwsb = lpool.tile([128, DC, E], F32R, tag="gwsb")
                    nc.gpsimd.dma_start(out=gwsb, in_=gw_d[l].rearrange("(c p) e -> p c e", p=128))
                    gbb = lpool.tile([128, E], F32, tag="gbb")
                    nc.gpsimd.dma_start(out=gbb, in_=bcast(gb_d[l], 128, E))

                    xnT = lpool.tile([128, DC, TPC], F32R, tag="xnT")

                    # ---- LN + transpose to feature-major ----
                    for g in range(G):
                        stats = small.tile([128, 2, 6], F32, tag="stats")
                        nc.vector.bn_stats(out=stats[:, 0, :], in_=h_t[:, g, 0:512])
                        nc.vector.bn_stats(out=stats[:, 1, :], in_=h_t[:, g, 512:1024])
                        mv = small.tile([128, 2], F32, tag="mv")
                        nc.vector.bn_aggr(out=mv, in_=stats)
                        sd = small.tile([128, 1], F32, tag="sd")
                        nc.scalar.activation(sd, mv[:, 1:2], AF.Sqrt, bias=eps_t)
                        rs = small.tile([128, 1], F32, tag="rs")
                        nc.vector.reciprocal(rs, sd)
                        xn = xnp.tile([128, D], F32, tag="xn")
                        nc.vector.tensor_scalar(
                            out=xn, in0=h_t[:, g, :],
                            scalar1=mv[:, 0:1], scalar2=rs,
                            op0=ALU.subtract, op1=ALU.mult,
                        )
                        for c in range(DC):
                            pt = ps_t.tile([128, 128], F32, tag="pt")
                            nc.tensor.transpose(pt, xn[:, c * 128 : (c + 1) * 128], ident)
                            nc.vector.tensor_scalar(
                                out=xnT[:, c, g * 128 : (g + 1) * 128],
                                in0=pt, scalar1=g_sb[:, c, :], scalar2=b_sb[:, c, :],
                                op0=ALU.mult, op1=ALU.add,
                            )

                    # ---- gating softmax (token-major) ----
                    for g in range(G):
                        plg = ps_lg.tile([128, E], F32, tag="plg")
                        for c in range(DC):
                            nc.tensor.matmul(
                                plg, xnT[:, c, g * 128 : (g + 1) * 128], gwsb[:, c, :],
                                start=(c == 0), stop=(c == DC - 1),
                            )
                        lg = small.tile([128, E], F32, tag="lg")
                        nc.vector.tensor_tensor(out=lg, in0=plg, in1=gbb, op=ALU.add)
                        nm = small.tile([128, 1], F32, tag="nm")
                        nc.vector.tensor_reduce(
                            out=nm, in_=lg, axis=mybir.AxisListType.X,
                            op=ALU.max, negate=True,
                        )
                        pexp = small.tile([128, E], F32, tag="pexp")
                        ssum = small.tile([128, 1], F32, tag="ssum")
                        nc.scalar.activation(pexp, lg, AF.Exp, bias=nm, accum_out=ssum)
                        rinv = small.tile([128, 1], F32, tag="rinv")
                        nc.vector.reciprocal(rinv, ssum)
                        nc.vector.tensor_scalar_mul(rw_t[:, g, :], pexp, rinv)

                    # ---- experts ----
                    for e in range(E):
                        for n in range(NTC):
                            e1g = lpool.tile([128, IC, 512], F32R, tag="e1g")
                            for m in range(IC):
                                w1blk = w1p.tile([128, DC, 128], F32R, tag="w1blk")
                                nc.sync.dma_start(
                                    out=w1blk,
                                    in_=w1_d[l, e, :, m * 128 : (m + 1) * 128]
                                    .rearrange("(c p) i -> p c i", p=128),
                                )
                                pe1 = ps_e1.tile([128, 512], F32, tag="pe1")
                                for c in range(DC):
                                    nc.tensor.matmul(
                                        pe1, w1blk[:, c, :],
                                        xnT[:, c, n * 512 : (n + 1) * 512],
                                        start=(c == 0), stop=(c == DC - 1),
                                    )
                                nc.scalar.activation(e1g[:, m, :], pe1, AF.Gelu)
                            for dc in range(DCB):
                                w2blk = w2p.tile([128, IC, 512], F32R, tag="w2blk")
                                nc.gpsimd.dma_start(
                                    out=w2blk,
                                    in_=w2_d[l, e, :, dc * 512 : (dc + 1) * 512]
                                    .rearrange("(c p) d -> p c d", p=128),
                                )
                                for tt in range(4):
                                    g = n * 4 + tt
                                    peo = ps_eo.tile([128, 512], F32, tag="peo")
                                    for c in range(IC):
                                        nc.tensor.matmul(
                                            peo, e1g[:, c, tt * 128 : (tt + 1) * 128],
                                            w2blk[:, c, :],
                                            start=(c == 0), stop=(c == IC - 1),
                                        )
                                    eo_sb = evacp.tile([128, 512], F32, tag="eo_sb")
                                    nc.scalar.copy(eo_sb, peo)
                                    nc.scalar.dma_start(
                                        out=experts_d[
                                            l, g * 128 : (g + 1) * 128, e,
                                            dc * 512 : (dc + 1) * 512,
                                        ],
                                        in_=eo_sb,
                                    )
                                    nc.vector.scalar_tensor_tensor(
                                        out=h_t[:, g, dc * 512 : (dc + 1) * 512],
                                        in0=peo, scalar=rw_t[:, g, e : e + 1],
                                        in1=h_t[:, g, dc * 512 : (dc + 1) * 512],
                                        op0=ALU.mult, op1=ALU.add,
                                    )

        # ---- head ----
        with nc.named_scope("head"):
            with ExitStack() as hctx:
                hpool = hctx.enter_context(tc.tile_pool(name="hpool", bufs=1))
                hwp = hctx.enter_context(tc.tile_pool(name="hwp", bufs=2))
                bbp = hctx.enter_context(tc.tile_pool(name="bbp", bufs=2))
                lgp = hctx.enter_context(tc.tile_pool(name="lgp", bufs=4))
                ps_lo = hctx.enter_context(tc.tile_pool(name="ps_lo", bufs=4, space="PSUM"))

                hT = hpool.tile([128, DC, TPC], F32R)
                for g in range(G):
                    for c in range(DC):
                        pt = ps_t.tile([128, 128], F32, tag="pt")
                        nc.tensor.transpose(pt, h_t[:, g, c * 128 : (c + 1) * 128], ident)
                        nc.vector.tensor_copy(hT[:, c, g * 128 : (g + 1) * 128], pt)

                for off, nv in VCHUNKS:
                    hwblk = hwp.tile([128, DC, 512], F32R, tag="hwblk")
                    nc.sync.dma_start(
                        out=hwblk[:, :, :nv],
                        in_=hw_d[:, off : off + nv].rearrange("(c p) v -> p c v", p=128),
                    )
                    bias_bc = bbp.tile([128, 512], F32, tag="bias_bc")
                    nc.gpsimd.dma_start(out=bias_bc[:, :nv], in_=bcast(hb_d[off : off + nv], 128, nv))
                    for g in range(G):
                        plo = ps_lo.tile([128, 512], F32, tag="plo")
                        for c in range(DC):
                            nc.tensor.matmul(
                                plo[:, :nv], hT[:, c, g * 128 : (g + 1) * 128],
                                hwblk[:, c, :nv],
                                start=(c == 0), stop=(c == DC - 1),
                            )
                        lg_sb = lgp.tile([128, 512], F32, tag="lg_sb")
                        nc.vector.tensor_tensor(
                            out=lg_sb[:, :nv], in0=plo[:, :nv], in1=bias_bc[:, :nv], op=ALU.add
                        )
                        nc.scalar.dma_start(
                            out=logits_d[g * 128 : (g + 1) * 128, off : off + nv],
                            in_=lg_sb[:, :nv],
                        )

    nc.compile()
    return nc


_NC_CACHE = None


def _get_nc():
    global _NC_CACHE
    if _NC_CACHE is None:
        _NC_CACHE = build()
    return _NC_CACHE


def kernel(x, emb, ln_g, ln_b, gate_w, gate_b, w1, w2, head_w, head_b):
    ids = np.ascontiguousarray(np.asarray(x).reshape(-1).astype(np.int32))
    emb = np.ascontiguousarray(np.asarray(emb, dtype=np.float32))
    ln_g = np.ascontiguousarray(np.asarray(ln_g, dtype=np.float32))
    ln_b = np.ascontiguousarray(np.asarray(ln_b, dtype=np.float32))
    gate_b = np.ascontiguousarray(np.asarray(gate_b, dtype=np.float32))
    head_b = np.ascontiguousarray(np.asarray(head_b, dtype=np.float32))
    gate_wr = round_f32r(np.asarray(gate_w))
    w1r = round_f32r(np.asarray(w1))
    w2r = round_f32r(np.asarray(w2))
    head_wr = round_f32r(np.asarray(head_w))

    nc = _get_nc()
    in_maps = []
    for c in range(NCORES):
        in_maps.append({
            "ids": ids[c * TPC : (c + 1) * TPC],
            "emb": emb,
            "ln_g": ln_g,
            "ln_b": ln_b,
            "gate_w": gate_wr,
            "gate_b": gate_b,
            "w1": w1r,
            "w2": w2r,
            "head_w": head_wr,
            "head_b": head_b,
        })
    r = run_bass_kernel_spmd(nc, in_maps, list(range(NCORES)))
    logits = np.concatenate(
        [r.results[c]["logits"] for c in range(NCORES)], axis=0
    ).reshape(B, T, VOCAB)
    experts = np.concatenate(
        [r.results[c]["experts"] for c in range(NCORES)], axis=1
    ).reshape(L, B, T, E, D)
    return logits, experts


# revision 13
# speedup vs baseline: 1.1013x; 1.1013x over previous
"""Trainium2 Bass kernel for a 2-layer dense-MoE PoC model.

Model (per reference):
  h = emb[x]                                         [B,T,D]
  for l in 0..L-1:
    xn = layernorm(h) * g + b
    rw = softmax(xn @ gate_w[l] + gate_b[l])         [B,T,E]
    e  = gelu_erf(einsum('btc,eci->btei', xn, w1[l]))
    eo = einsum('btei,eic->btec', e, w2[l])
    h  = h + einsum('btec,bte->btc', eo, rw)
  logits = h @ head_w + head_b
  return logits, stack(eo per layer)

Shapes: VOCAB=32000 D=1024 E=8 I=2048 L=2 B=4 T=2048.

Distribution: data-parallel over the 8192 tokens across 8 NeuronCores
(1024 tokens each); every core holds the full weights.  No collectives.

Numerics: all large matmuls run in float32r (fp32 rounded to an 11-bit
mantissa; full PE rate).  Weights are pre-rounded on host and staged as
f32r; activations are rounded on-device by the PSUM-evacuation copies
that produce the matmul operands.  Everything else is fp32.
"""
from contextlib import ExitStack

import numpy as np

import concourse.bass as bass
import concourse.mybir as mybir
import concourse.tile as tile
from concourse import bacc
from concourse.bass_utils import run_bass_kernel_spmd

# model dims (hardcoded per contract)
VOCAB, D, E, I, L, B, T = 32000, 1024, 8, 2048, 2, 4, 2048
LN_EPS = 1e-5
NCORES = 8
TOK = B * T              # 8192
TPC = TOK // NCORES      # 1024 tokens per core
G = TPC // 128           # 8 token tiles per core
DC = D // 128            # 8 feature chunks
IC = I // 128            # 16 expert-hidden chunks
NTC = TPC // 512         # 2 moving chunks of 512 tokens
DCB = D // 512           # 2 output D chunks of 512

F32 = mybir.dt.float32
F32R = mybir.dt.float32r
I32 = mybir.dt.int32
AF = mybir.ActivationFunctionType
ALU = mybir.AluOpType

# vocab chunks for the head: 62 x 512 + 1 x 256
VCHUNKS = [(i * 512, 512) for i in range(VOCAB // 512)]
if VOCAB % 512:
    VCHUNKS.append((512 * (VOCAB // 512), VOCAB % 512))


def round_f32r(x: np.ndarray) -> np.ndarray:
    """Round-to-nearest fp32 -> f32r (11 explicit mantissa bits)."""
    x = np.ascontiguousarray(x, dtype=np.float32)
    b = x.view(np.uint32)
    r = (b + np.uint32(0x800) + ((b >> np.uint32(12)) & np.uint32(1))) & np.uint32(0xFFFFF000)
    return r.view(np.float32)


def build():
    from concourse.masks import make_identity

    nc = bacc.Bacc("TRN2", target_bir_lowering=False, debug=False)

    ids_d = nc.declare_dram_parameter("ids", [TPC], I32, isOutput=False)
    emb_d = nc.declare_dram_parameter("emb", [VOCAB, D], F32, isOutput=False)
    lng_d = nc.declare_dram_parameter("ln_g", [L, D], F32, isOutput=False)
    lnb_d = nc.declare_dram_parameter("ln_b", [L, D], F32, isOutput=False)
    gw_d = nc.declare_dram_parameter("gate_w", [L, D, E], F32R, isOutput=False)
    gb_d = nc.declare_dram_parameter("gate_b", [L, E], F32, isOutput=False)
    # w1/w2 are staged pre-packed so each [128, ...] tile DMA reads
    # per-partition-contiguous blocks (4KB / 32KB descriptors).
    w1_d = nc.declare_dram_parameter("w1", [L, E, IC, 128, DC, 128], F32R, isOutput=False)
    w2_d = nc.declare_dram_parameter("w2", [L, E, DCB, 128, IC, 512], F32R, isOutput=False)
    hw_d = nc.declare_dram_parameter("head_w", [D, VOCAB], F32R, isOutput=False)
    hb_d = nc.declare_dram_parameter("head_b", [VOCAB], F32, isOutput=False)
    logits_d = nc.declare_dram_parameter("logits", [TPC, VOCAB], F32, isOutput=True)
    experts_d = nc.declare_dram_parameter("experts", [L, TPC, E, D], F32, isOutput=True)

    def bcast(dram_ap, parts, n):
        """DRAM AP replicating a length-n row across `parts` partitions."""
        return bass.AP(
            tensor=dram_ap.tensor,
            offset=dram_ap.offset,
            ap=[[0, parts]] + [list(p) for p in dram_ap.ap],
        )

    with ExitStack() as ctx:
        tc = ctx.enter_context(tile.TileContext(nc))

        persist = ctx.enter_context(tc.tile_pool(name="persist", bufs=1))
        small = ctx.enter_context(tc.tile_pool(name="small", bufs=4))
        ps_t = ctx.enter_context(tc.tile_pool(name="ps_t", bufs=2, space="PSUM"))

        ident = persist.tile([128, 128], F32)
        make_identity(nc, ident)
        eps_t = persist.tile([128, 1], F32)
        nc.vector.memset(eps_t, LN_EPS)

        h_t = persist.tile([128, G, D], F32)
        rw_t = persist.tile([128, G, E], F32)

        # ---- embedding gather ----
        with nc.named_scope("embed"):
            ids_t = persist.tile([128, G], I32)
            nc.sync.dma_start(out=ids_t, in_=ids_d.rearrange("(g p) -> p g", p=128))
            for g in range(G):
                nc.gpsimd.indirect_dma_start(
                    out=h_t[:, g, :],
                    out_offset=None,
                    in_=emb_d[:],
                    in_offset=bass.IndirectOffsetOnAxis(ap=ids_t[:, g : g + 1], axis=0),
                )

        with ExitStack() as lctx:
            lpool = lctx.enter_context(tc.tile_pool(name="lpool", bufs=1))
            xnp = lctx.enter_context(tc.tile_pool(name="xnp", bufs=2))
            w1p = lctx.enter_context(tc.tile_pool(name="w1p", bufs=2))
            w2p = lctx.enter_context(tc.tile_pool(name="w2p", bufs=2))
            evacp = lctx.enter_context(tc.tile_pool(name="evacp", bufs=3))
            ps_lg = lctx.enter_context(tc.tile_pool(name="ps_lg", bufs=1, space="PSUM"))
            ps_e1 = lctx.enter_context(tc.tile_pool(name="ps_e1", bufs=3, space="PSUM"))
            ps_eo = lctx.enter_context(tc.tile_pool(name="ps_eo", bufs=2, space="PSUM"))

            for l in range(L):
                with nc.named_scope(f"layer{l}"):
                    # per-layer params
                    g_sb = lpool.tile([128, DC, 1], F32, tag="g_sb")
                    b_sb = lpool.tile([128, DC, 1], F32, tag="b_sb")
                    nc.gpsimd.dma_start(out=g_sb, in_=lng_d[l].rearrange("(c p) -> p c ()", p=128))
                    nc.gpsimd.dma_start(out=b_sb, in_=lnb_d[l].rearrange("(c p) -> p c ()", p=128))
                    gwsb = lpool.tile([128, DC, E], F32R, tag="gwsb")
                    nc.gpsimd.dma_start(out=gwsb, in_=gw_d[l].rearrange("(c p) e -> p c e", p=128))
                    gbb = lpool.tile([128, E], F32, tag="gbb")
                    nc.gpsimd.dma_start(out=gbb, in_=bcast(gb_d[l], 128, E))

                    xnT = lpool.tile([128, DC, TPC], F32R, tag="xnT")

                    # ---- LN + transpose to feature-major ----
                    for g in range(G):
                        stats = small.tile([128, 2, 6], F32, tag="stats")
                        nc.vector.bn_stats(out=stats[:, 0, :], in_=h_t[:, g, 0:512])
                        nc.vector.bn_stats(out=stats[:, 1, :], in_=h_t[:, g, 512:1024])
                        mv = small.tile([128, 2], F32, tag="mv")
                        nc.vector.bn_aggr(out=mv, in_=stats)
                        sd = small.tile([128, 1], F32, tag="sd")
                        nc.scalar.activation(sd, mv[:, 1:2], AF.Sqrt, bias=eps_t)
                        rs = small.tile([128, 1], F32, tag="rs")
                        nc.vector.reciprocal(rs, sd)
                        xn = xnp.tile([128, D], F32, tag="xn")
                        nc.vector.tensor_scalar(
                            out=xn, in0=h_t[:, g, :],
                            scalar1=mv[:, 0:1], scalar2=rs,
                            op0=ALU.subtract, op1=ALU.mult,
                        )
                        for c in range(DC):
                            pt = ps_t.tile([128, 128], F32, tag="pt")
                            nc.tensor.transpose(pt, xn[:, c * 128 : (c + 1) * 128], ident)
                            nc.vector.tensor_scalar(
                                out=xnT[:, c, g * 128 : (g + 1) * 128],
                                in0=pt, scalar1=g_sb[:, c, :], scalar2=b_sb[:, c, :],
                                op0=ALU.mult, op1=ALU.add,
                            )

                    # ---- gating softmax (token-major) ----
                    for g in range(G):
                        plg = ps_lg.tile([128, E], F32, tag="plg")
                        for c in range(DC):
                            nc.tensor.matmul(
                                plg, xnT[:, c, g * 128 : (g + 1) * 128], gwsb[:, c, :],
                                start=(c == 0), stop=(c == DC - 1),
                            )
                        lg = small.tile([128, E], F32, tag="lg")
                        nc.vector.tensor_tensor(out=lg, in0=plg, in1=gbb, op=ALU.add)
                        nm = small.tile([128, 1], F32, tag="nm")
                        nc.vector.tensor_reduce(
                            out=nm, in_=lg, axis=mybir.AxisListType.X,
                            op=ALU.max, negate=True,
                        )
                        pexp = small.tile([128, E], F32, tag="pexp")
                        ssum = small.tile([128, 1], F32, tag="ssum")
                        nc.scalar.activation(pexp, lg, AF.Exp, bias=nm, accum_out=ssum)
                        rinv = small.tile([128, 1], F32, tag="rinv")
                        nc.vector.reciprocal(rinv, ssum)
                        nc.vector.tensor_scalar_mul(rw_t[:, g, :], pexp, rinv)

                    # ---- experts ----
                    for e in range(E):
                        for n in range(NTC):
                            e1g = lpool.tile([128, IC, 512], F32R, tag="e1g")
                            for m in range(IC):
                                w1blk = w1p.tile([128, DC, 128], F32R, tag="w1blk")
                                nc.sync.dma_start(out=w1blk, in_=w1_d[l, e, m])
                                pe1 = ps_e1.tile([128, 512], F32, tag="pe1")
                                for c in range(DC):
                                    nc.tensor.matmul(
                                        pe1, w1blk[:, c, :],
                                        xnT[:, c, n * 512 : (n + 1) * 512],
                                        start=(c == 0), stop=(c == DC - 1),
                                    )
                                nc.scalar.activation(e1g[:, m, :], pe1, AF.Gelu)
                            for dc in range(DCB):
                                w2blk = w2p.tile([128, IC, 512], F32R, tag="w2blk")
                                nc.sync.dma_start(out=w2blk, in_=w2_d[l, e, dc])
                                for tt in range(4):
                                    g = n * 4 + tt
                                    peo = ps_eo.tile([128, 512], F32, tag="peo")
                                    for c in range(IC):
                                        nc.tensor.matmul(
                                            peo, e1g[:, c, tt * 128 : (tt + 1) * 128],
                                            w2blk[:, c, :],
                                            start=(c == 0), stop=(c == IC - 1),
                                        )
                                    eo_sb = evacp.tile([128, 512], F32, tag="eo_sb")
                                    nc.scalar.copy(eo_sb, peo)
                                    nc.sync.dma_start(
                                        out=experts_d[
                                            l, g * 128 : (g + 1) * 128, e,
                                            dc * 512 : (dc + 1) * 512,
                                        ],
                                        in_=eo_sb,
                                    )
                                    nc.vector.scalar_tensor_tensor(
                                        out=h_t[:, g, dc * 512 : (dc + 1) * 512],
                                        in0=peo, scalar=rw_t[:, g, e : e + 1],
                                        in1=h_t[:, g, dc * 512 : (dc + 1) * 512],
                                        op0=ALU.mult, op1=ALU.add,
                                    )

        # ---- head ----
        with nc.named_scope("head"):
            with ExitStack() as hctx:
                hpool = hctx.enter_context(tc.tile_pool(name="hpool", bufs=1))
                hwp = hctx.enter_context(tc.tile_pool(name="hwp", bufs=2))
                bbp = hctx.enter_context(tc.tile_pool(name="bbp", bufs=2))
                lgp = hctx.enter_context(tc.tile_pool(name="lgp", bufs=4))
                ps_lo = hctx.enter_context(tc.tile_pool(name="ps_lo", bufs=4, space="PSUM"))

                hT = hpool.tile([128, DC, TPC], F32R)
                for g in range(G):
                    for c in range(DC):
                        pt = ps_t.tile([128, 128], F32, tag="pt")
                        nc.tensor.transpose(pt, h_t[:, g, c * 128 : (c + 1) * 128], ident)
                        nc.vector.tensor_copy(hT[:, c, g * 128 : (g + 1) * 128], pt)

                for off, nv in VCHUNKS:
                    hwblk = hwp.tile([128, DC, 512], F32R, tag="hwblk")
                    nc.sync.dma_start(
                        out=hwblk[:, :, :nv],
                        in_=hw_d[:, off : off + nv].rearrange("(c p) v -> p c v", p=128),
                    )
                    bias_bc = bbp.tile([128, 512], F32, tag="bias_bc")
                    nc.gpsimd.dma_start(out=bias_bc[:, :nv], in_=bcast(hb_d[off : off + nv], 128, nv))
                    for g in range(G):
                        plo = ps_lo.tile([128, 512], F32, tag="plo")
                        for c in range(DC):
                            nc.tensor.matmul(
                                plo[:, :nv], hT[:, c, g * 128 : (g + 1) * 128],
                                hwblk[:, c, :nv],
                                start=(c == 0), stop=(c == DC - 1),
                            )
                        lg_sb = lgp.tile([128, 512], F32, tag="lg_sb")
                        nc.vector.tensor_tensor(
                            out=lg_sb[:, :nv], in0=plo[:, :nv], in1=bias_bc[:, :nv], op=ALU.add
                        )
                        nc.sync.dma_start(
                            out=logits_d[g * 128 : (g + 1) * 128, off : off + nv],
                            in_=lg_sb[:, :nv],
                        )

    nc.compile()
    return nc


_NC_CACHE = None


def _get_nc():
    global _NC_CACHE
    if _NC_CACHE is None:
        _NC_CACHE = build()
    return _NC_CACHE


def kernel(x, emb, ln_g, ln_b, gate_w, gate_b, w1, w2, head_w, head_b):
    ids = np.ascontiguousarray(np.asarray(x).reshape(-1).astype(np.int32))
    emb = np.ascontiguousarray(np.asarray(emb, dtype=np.float32))
    ln_g = np.ascontiguousarray(np.asarray(ln_g, dtype=np.float32))
    ln_b = np.ascontiguousarray(np.asarray(ln_b, dtype=np.float32))
    gate_b = np.ascontiguousarray(np.asarray(gate_b, dtype=np.float32))
    head_b = np.ascontiguousarray(np.asarray(head_b, dtype=np.float32))
    gate_wr = round_f32r(np.asarray(gate_w))
    # pack to per-partition-contiguous tiled layouts (see build())
    w1r = np.ascontiguousarray(
        round_f32r(np.asarray(w1))
        .reshape(L, E, DC, 128, IC, 128)
        .transpose(0, 1, 4, 3, 2, 5)
    )
    w2r = np.ascontiguousarray(
        round_f32r(np.asarray(w2))
        .reshape(L, E, IC, 128, DCB, 512)
        .transpose(0, 1, 4, 3, 2, 5)
    )
    head_wr = round_f32r(np.asarray(head_w))

    nc = _get_nc()
    in_maps = []
    for c in range(NCORES):
        in_maps.append({
            "ids": ids[c * TPC : (c + 1) * TPC],
            "emb": emb,
            "ln_g": ln_g,
            "ln_b": ln_b,
            "gate_w": gate_wr,
            "gate_b": gate_b,
            "w1": w1r,
            "w2": w2r,
            "head_w": head_wr,
            "head_b": head_b,
        })
    r = run_bass_kernel_spmd(nc, in_maps, list(range(NCORES)))
    logits = np.concatenate(
        [r.results[c]["logits"] for c in range(NCORES)], axis=0
    ).reshape(B, T, VOCAB)
    experts = np.concatenate(
        [r.results[c]["experts"] for c in range(NCORES)], axis=1
    ).reshape(L, B, T, E, D)
    return logits, experts


# revision 16
# speedup vs baseline: 1.1845x; 1.0755x over previous
"""Trainium2 Bass kernel for a 2-layer dense-MoE PoC model.

Model (per reference):
  h = emb[x]                                         [B,T,D]
  for l in 0..L-1:
    xn = layernorm(h) * g + b
    rw = softmax(xn @ gate_w[l] + gate_b[l])         [B,T,E]
    e  = gelu_erf(einsum('btc,eci->btei', xn, w1[l]))
    eo = einsum('btei,eic->btec', e, w2[l])
    h  = h + einsum('btec,bte->btc', eo, rw)
  logits = h @ head_w + head_b
  return logits, stack(eo per layer)

Shapes: VOCAB=32000 D=1024 E=8 I=2048 L=2 B=4 T=2048.

Distribution: data-parallel over the 8192 tokens across 8 NeuronCores
(1024 tokens each); every core holds the full weights.  No collectives.

Numerics: all large matmuls run in float32r (fp32 rounded to an 11-bit
mantissa; full PE rate).  Weights are pre-rounded on host and staged as
f32r; activations are rounded on-device by the PSUM-evacuation copies
that produce the matmul operands.  Everything else is fp32.
"""
from contextlib import ExitStack

import numpy as np

import concourse.bass as bass
import concourse.mybir as mybir
import concourse.tile as tile
from concourse import bacc
from concourse.bass_utils import run_bass_kernel_spmd

# model dims (hardcoded per contract)
VOCAB, D, E, I, L, B, T = 32000, 1024, 8, 2048, 2, 4, 2048
LN_EPS = 1e-5
NCORES = 8
TOK = B * T              # 8192
TPC = TOK // NCORES      # 1024 tokens per core
G = TPC // 128           # 8 token tiles per core
DC = D // 128            # 8 feature chunks
IC = I // 128            # 16 expert-hidden chunks
NTC = TPC // 512         # 2 moving chunks of 512 tokens
DCB = D // 512           # 2 output D chunks of 512

F32 = mybir.dt.float32
F32R = mybir.dt.float32r
I32 = mybir.dt.int32
AF = mybir.ActivationFunctionType
ALU = mybir.AluOpType

# vocab chunks for the head: 62 x 512 + 1 x 256
VCHUNKS = [(i * 512, 512) for i in range(VOCAB // 512)]
if VOCAB % 512:
    VCHUNKS.append((512 * (VOCAB // 512), VOCAB % 512))


def round_f32r(x: np.ndarray) -> np.ndarray:
    """Round-to-nearest fp32 -> f32r (11 explicit mantissa bits)."""
    x = np.ascontiguousarray(x, dtype=np.float32)
    b = x.view(np.uint32)
    r = (b + np.uint32(0x800) + ((b >> np.uint32(12)) & np.uint32(1))) & np.uint32(0xFFFFF000)
    return r.view(np.float32)


def build():
    from concourse.masks import make_identity

    nc = bacc.Bacc("TRN2", target_bir_lowering=False, debug=False)

    ids_d = nc.declare_dram_parameter("ids", [TPC], I32, isOutput=False)
    emb_d = nc.declare_dram_parameter("emb", [VOCAB, D], F32, isOutput=False)
    lng_d = nc.declare_dram_parameter("ln_g", [L, D], F32, isOutput=False)
    lnb_d = nc.declare_dram_parameter("ln_b", [L, D], F32, isOutput=False)
    gw_d = nc.declare_dram_parameter("gate_w", [L, D, E], F32R, isOutput=False)
    gb_d = nc.declare_dram_parameter("gate_b", [L, E], F32, isOutput=False)
    # w1/w2 are staged pre-packed so each [128, ...] tile DMA reads
    # per-partition-contiguous blocks (4KB / 32KB descriptors).
    w1_d = nc.declare_dram_parameter("w1", [L, E, IC, 128, DC, 128], F32R, isOutput=False)
    w2_d = nc.declare_dram_parameter("w2", [L, E, DCB, 128, IC, 512], F32R, isOutput=False)
    hw_d = nc.declare_dram_parameter("head_w", [D, VOCAB], F32R, isOutput=False)
    hb_d = nc.declare_dram_parameter("head_b", [VOCAB], F32, isOutput=False)
    logits_d = nc.declare_dram_parameter("logits", [TPC, VOCAB], F32, isOutput=True)
    experts_d = nc.declare_dram_parameter("experts", [L, TPC, E, D], F32, isOutput=True)

    def bcast(dram_ap, parts, n):
        """DRAM AP replicating a length-n row across `parts` partitions."""
        return bass.AP(
            tensor=dram_ap.tensor,
            offset=dram_ap.offset,
            ap=[[0, parts]] + [list(p) for p in dram_ap.ap],
        )

    with ExitStack() as ctx:
        tc = ctx.enter_context(tile.TileContext(nc))

        persist = ctx.enter_context(tc.tile_pool(name="persist", bufs=1))
        small = ctx.enter_context(tc.tile_pool(name="small", bufs=4))
        ps_t = ctx.enter_context(tc.tile_pool(name="ps_t", bufs=2, space="PSUM"))

        ident = persist.tile([128, 128], F32)
        make_identity(nc, ident)
        eps_t = persist.tile([128, 1], F32)
        nc.vector.memset(eps_t, LN_EPS)

        h_t = persist.tile([128, G, D], F32)
        rw_t = persist.tile([128, G, E], F32)

        # ---- embedding gather ----
        with nc.named_scope("embed"):
            ids_t = persist.tile([128, G], I32)
            nc.sync.dma_start(out=ids_t, in_=ids_d.rearrange("(g p) -> p g", p=128))
            for g in range(G):
                nc.gpsimd.indirect_dma_start(
                    out=h_t[:, g, :],
                    out_offset=None,
                    in_=emb_d[:],
                    in_offset=bass.IndirectOffsetOnAxis(ap=ids_t[:, g : g + 1], axis=0),
                )

        with ExitStack() as lctx:
            lpool = lctx.enter_context(tc.tile_pool(name="lpool", bufs=1))
            xnp = lctx.enter_context(tc.tile_pool(name="xnp", bufs=2))
            w1p = lctx.enter_context(tc.tile_pool(name="w1p", bufs=3))
            w2p = lctx.enter_context(tc.tile_pool(name="w2p", bufs=2))
            evacp = lctx.enter_context(tc.tile_pool(name="evacp", bufs=2))
            ps_lg = lctx.enter_context(tc.tile_pool(name="ps_lg", bufs=1, space="PSUM"))
            ps_e1 = lctx.enter_context(tc.tile_pool(name="ps_e1", bufs=3, space="PSUM"))
            ps_eo = lctx.enter_context(tc.tile_pool(name="ps_eo", bufs=2, space="PSUM"))

            for l in range(L):
                with nc.named_scope(f"layer{l}"):
                    # per-layer params
                    g_sb = lpool.tile([128, DC, 1], F32, tag="g_sb")
                    b_sb = lpool.tile([128, DC, 1], F32, tag="b_sb")
                    nc.gpsimd.dma_start(out=g_sb, in_=lng_d[l].rearrange("(c p) -> p c ()", p=128))
                    nc.gpsimd.dma_start(out=b_sb, in_=lnb_d[l].rearrange("(c p) -> p c ()", p=128))
                    gwsb = lpool.tile([128, DC, E], F32R, tag="gwsb")
                    nc.gpsimd.dma_start(out=gwsb, in_=gw_d[l].rearrange("(c p) e -> p c e", p=128))
                    gbb = lpool.tile([128, E], F32, tag="gbb")
                    nc.gpsimd.dma_start(out=gbb, in_=bcast(gb_d[l], 128, E))

                    xnT = lpool.tile([128, DC, TPC], F32R, tag="xnT")

                    # ---- LN + transpose to feature-major ----
                    for g in range(G):
                        stats = small.tile([128, 2, 6], F32, tag="stats")
                        nc.vector.bn_stats(out=stats[:, 0, :], in_=h_t[:, g, 0:512])
                        nc.vector.bn_stats(out=stats[:, 1, :], in_=h_t[:, g, 512:1024])
                        mv = small.tile([128, 2], F32, tag="mv")
                        nc.vector.bn_aggr(out=mv, in_=stats)
                        sd = small.tile([128, 1], F32, tag="sd")
                        nc.scalar.activation(sd, mv[:, 1:2], AF.Sqrt, bias=eps_t)
                        rs = small.tile([128, 1], F32, tag="rs")
                        nc.vector.reciprocal(rs, sd)
                        xn = xnp.tile([128, D], F32, tag="xn")
                        nc.vector.tensor_scalar(
                            out=xn, in0=h_t[:, g, :],
                            scalar1=mv[:, 0:1], scalar2=rs,
                            op0=ALU.subtract, op1=ALU.mult,
                        )
                        for c in range(DC):
                            pt = ps_t.tile([128, 128], F32, tag="pt")
                            nc.tensor.transpose(pt, xn[:, c * 128 : (c + 1) * 128], ident)
                            nc.vector.tensor_scalar(
                                out=xnT[:, c, g * 128 : (g + 1) * 128],
                                in0=pt, scalar1=g_sb[:, c, :], scalar2=b_sb[:, c, :],
                                op0=ALU.mult, op1=ALU.add,
                            )

                    # ---- gating softmax (token-major) ----
                    for g in range(G):
                        plg = ps_lg.tile([128, E], F32, tag="plg")
                        for c in range(DC):
                            nc.tensor.matmul(
                                plg, xnT[:, c, g * 128 : (g + 1) * 128], gwsb[:, c, :],
                                start=(c == 0), stop=(c == DC - 1),
                            )
                        lg = small.tile([128, E], F32, tag="lg")
                        nc.vector.tensor_tensor(out=lg, in0=plg, in1=gbb, op=ALU.add)
                        nm = small.tile([128, 1], F32, tag="nm")
                        nc.vector.tensor_reduce(
                            out=nm, in_=lg, axis=mybir.AxisListType.X,
                            op=ALU.max, negate=True,
                        )
                        pexp = small.tile([128, E], F32, tag="pexp")
                        ssum = small.tile([128, 1], F32, tag="ssum")
                        nc.scalar.activation(pexp, lg, AF.Exp, bias=nm, accum_out=ssum)
                        rinv = small.tile([128, 1], F32, tag="rinv")
                        nc.vector.reciprocal(rinv, ssum)
                        nc.vector.tensor_scalar_mul(rw_t[:, g, :], pexp, rinv)

                    # ---- experts ----
                    for e in range(E):
                        for n in range(NTC):
                            e1g = lpool.tile([128, IC, 512], F32R, tag="e1g")
                            # w2 blocks are issued mid-way through the w1
                            # m-loop so the FIFO DMA ring has them in flight
                            # before the w2 phase needs them.
                            w2blks = [None, None]
                            for m in range(IC):
                                w1blk = w1p.tile([128, DC, 128], F32R, tag="w1blk")
                                nc.sync.dma_start(out=w1blk, in_=w1_d[l, e, m])
                                if m == 4 or m == 10:
                                    dc = 0 if m == 4 else 1
                                    w2blks[dc] = w2p.tile([128, IC, 512], F32R, tag="w2blk", name="w2blk")
                                    nc.sync.dma_start(out=w2blks[dc], in_=w2_d[l, e, dc])
                                pe1 = ps_e1.tile([128, 512], F32, tag="pe1")
                                for c in range(DC):
                                    nc.tensor.matmul(
                                        pe1, w1blk[:, c, :],
                                        xnT[:, c, n * 512 : (n + 1) * 512],
                                        start=(c == 0), stop=(c == DC - 1),
                                    )
                                nc.scalar.activation(e1g[:, m, :], pe1, AF.Gelu)
                            for dc in range(DCB):
                                w2blk = w2blks[dc]
                                for tt in range(4):
                                    g = n * 4 + tt
                                    peo = ps_eo.tile([128, 512], F32, tag="peo")
                                    for c in range(IC):
                                        nc.tensor.matmul(
                                            peo, e1g[:, c, tt * 128 : (tt + 1) * 128],
                                            w2blk[:, c, :],
                                            start=(c == 0), stop=(c == IC - 1),
                                        )
                                    eo_sb = evacp.tile([128, 512], F32, tag="eo_sb")
                                    nc.scalar.copy(eo_sb, peo)
                                    nc.sync.dma_start(
                                        out=experts_d[
                                            l, g * 128 : (g + 1) * 128, e,
                                            dc * 512 : (dc + 1) * 512,
                                        ],
                                        in_=eo_sb,
                                    )
                                    nc.vector.scalar_tensor_tensor(
                                        out=h_t[:, g, dc * 512 : (dc + 1) * 512],
                                        in0=peo, scalar=rw_t[:, g, e : e + 1],
                                        in1=h_t[:, g, dc * 512 : (dc + 1) * 512],
                                        op0=ALU.mult, op1=ALU.add,
                                    )

        # ---- head ----
        with nc.named_scope("head"):
            with ExitStack() as hctx:
                hpool = hctx.enter_context(tc.tile_pool(name="hpool", bufs=1))
                hwp = hctx.enter_context(tc.tile_pool(name="hwp", bufs=2))
                bbp = hctx.enter_context(tc.tile_pool(name="bbp", bufs=2))
                lgp = hctx.enter_context(tc.tile_pool(name="lgp", bufs=4))
                ps_lo = hctx.enter_context(tc.tile_pool(name="ps_lo", bufs=4, space="PSUM"))

                hT = hpool.tile([128, DC, TPC], F32R)
                for g in range(G):
                    for c in range(DC):
                        pt = ps_t.tile([128, 128], F32, tag="pt")
                        nc.tensor.transpose(pt, h_t[:, g, c * 128 : (c + 1) * 128], ident)
                        nc.vector.tensor_copy(hT[:, c, g * 128 : (g + 1) * 128], pt)

                for off, nv in VCHUNKS:
                    hwblk = hwp.tile([128, DC, 512], F32R, tag="hwblk")
                    nc.sync.dma_start(
                        out=hwblk[:, :, :nv],
                        in_=hw_d[:, off : off + nv].rearrange("(c p) v -> p c v", p=128),
                    )
                    bias_bc = bbp.tile([128, 512], F32, tag="bias_bc")
                    nc.gpsimd.dma_start(out=bias_bc[:, :nv], in_=bcast(hb_d[off : off + nv], 128, nv))
                    for g in range(G):
                        plo = ps_lo.tile([128, 512], F32, tag="plo")
                        for c in range(DC):
                            nc.tensor.matmul(
                                plo[:, :nv], hT[:, c, g * 128 : (g + 1) * 128],
                                hwblk[:, c, :nv],
                                start=(c == 0), stop=(c == DC - 1),
                            )
                        lg_sb = lgp.tile([128, 512], F32, tag="lg_sb")
                        nc.vector.tensor_tensor(
                            out=lg_sb[:, :nv], in0=plo[:, :nv], in1=bias_bc[:, :nv], op=ALU.add
                        )
                        nc.sync.dma_start(
                            out=logits_d[g * 128 : (g + 1) * 128, off : off + nv],
                            in_=lg_sb[:, :nv],
                        )

    nc.compile()
    return nc


_NC_CACHE = None


def _get_nc():
    global _NC_CACHE
    if _NC_CACHE is None:
        _NC_CACHE = build()
    return _NC_CACHE


def kernel(x, emb, ln_g, ln_b, gate_w, gate_b, w1, w2, head_w, head_b):
    ids = np.ascontiguousarray(np.asarray(x).reshape(-1).astype(np.int32))
    emb = np.ascontiguousarray(np.asarray(emb, dtype=np.float32))
    ln_g = np.ascontiguousarray(np.asarray(ln_g, dtype=np.float32))
    ln_b = np.ascontiguousarray(np.asarray(ln_b, dtype=np.float32))
    gate_b = np.ascontiguousarray(np.asarray(gate_b, dtype=np.float32))
    head_b = np.ascontiguousarray(np.asarray(head_b, dtype=np.float32))
    gate_wr = round_f32r(np.asarray(gate_w))
    # pack to per-partition-contiguous tiled layouts (see build())
    w1r = np.ascontiguousarray(
        round_f32r(np.asarray(w1))
        .reshape(L, E, DC, 128, IC, 128)
        .transpose(0, 1, 4, 3, 2, 5)
    )
    w2r = np.ascontiguousarray(
        round_f32r(np.asarray(w2))
        .reshape(L, E, IC, 128, DCB, 512)
        .transpose(0, 1, 4, 3, 2, 5)
    )
    head_wr = round_f32r(np.asarray(head_w))

    nc = _get_nc()
    in_maps = []
    for c in range(NCORES):
        in_maps.append({
            "ids": ids[c * TPC : (c + 1) * TPC],
            "emb": emb,
            "ln_g": ln_g,
            "ln_b": ln_b,
            "gate_w": gate_wr,
            "gate_b": gate_b,
            "w1": w1r,
            "w2": w2r,
            "head_w": head_wr,
            "head_b": head_b,
        })
    r = run_bass_kernel_spmd(nc, in_maps, list(range(NCORES)))
    logits = np.concatenate(
        [r.results[c]["logits"] for c in range(NCORES)], axis=0
    ).reshape(B, T, VOCAB)
    experts = np.concatenate(
        [r.results[c]["experts"] for c in range(NCORES)], axis=1
    ).reshape(L, B, T, E, D)
    return logits, experts
